# revision 15
# baseline (speedup 1.0000x reference)
"""Top-1 MoE mapper kernel for Trainium2, SPMD over 8 NeuronCores.

Problem (hardcoded shapes):
  x  [2048, 1, 1024] f32   token inputs
  t  [2048, 8, 4096] f32   gating context
  W  [12, 1024, 4096] f32  expert weights
  b  [12, 4096] f32        expert biases
  Wg [4096, 12] f32        gate weights
  bg [12] f32              gate bias
  out[b] = x[b] @ W[argmax(t[b].mean(T) @ Wg + bg)] + b[...]  -> [2048, 1, 4096]

Strategy (v3):
  - Gating data-parallel over B (each core reads its 256-token slice of t as
    16 x 2MB chunks alternating across the two HWDGE queues with a 4-deep
    chunk pool so the stream never stalls on consumption; tree-reduce over T
    spread across DVE+ACT; PE transposes; f32 gate matmul; argmax). Gating is
    f32 end-to-end so the device top-1 matches the reference exactly.
  - ONE AllGather of the 256 local top-1 ids -> all 2048 ids on every core.
  - Routing batched across all 16 token tiles: one-hots, counts via one
    matmul, tile-base prefix via one [16,16] matmul, within-tile rank via one
    lsl matmul + per-tile base-broadcast matmuls; 16 back-to-back indirect
    scatters build the slot->token table. The table is sentinel-initialized
    so capacity-padded slots drop their gather traffic (bounds_check with
    oob_is_err=False).
  - Expert matmul output-column-parallel: core c holds W[:, :, c*512:(c+1)*512]
    cast to bf16 during the prefetch DMA (SWDGE). W streams through a 4-slot
    pool whose first 4 slots are held by dummy tiles released by late gating
    chunks, so the t stream owns the HBM pipe early and W fills the
    routing/compute window. Per expert: 2 x 128-row indirect gathers of x,
    PE transposes, bf16 matmuls (N=512), f32 bias via K=1 matmul, bf16
    results written contiguously in slot order on the idle HWDGE queue.
  - Host unpermutes slots -> tokens using the device-computed top-1 ids
    (pure data movement; all routing math happens on device).
"""

import numpy as np

import concourse.bass as bass
import concourse.bacc as bacc
import concourse.mybir as mybir
import concourse.tile as tile
from concourse.bass import IndirectOffsetOnAxis
from concourse.bass_utils import run_bass_kernel_spmd

F32 = mybir.dt.float32
F32R = mybir.dt.float32r
BF16 = mybir.dt.bfloat16
U32 = mybir.dt.uint32

B, T, IN, OUT, E = 2048, 8, 1024, 4096, 12
NCORES = 8
BS = B // NCORES            # 256 tokens per core (gating shard)
CS = OUT // NCORES          # 512 output columns per core (expert shard)
CAP = 256                   # capacity slots per expert
SLOTS = E * CAP             # 3072
NT = B // 128               # 16 token tiles globally
NTT = BS // 128             # 2 token tiles per core
NKX = IN // 128             # 8 k-tiles over the expert contraction
DC = 512                    # gating d-chunk width
NDC = OUT // DC             # 8 chunks per token tile
NCH = NTT * NDC             # 16 chunks total
SENTINEL = 3000000000.0     # > B-1 as u32 -> dropped by bounds_check


def build_kernel(enable_asserts: bool = False):
    nc = bacc.Bacc(
        "TRN2",
        target_bir_lowering=False,
        debug=False,
        enable_asserts=enable_asserts,
        num_devices=NCORES,
    )

    # ---- I/O -------------------------------------------------------------
    t_sh = nc.dram_tensor("t_sh", [BS, T, OUT], F32, kind="ExternalInput")
    x_full = nc.dram_tensor("x_full", [B, IN], F32R, kind="ExternalInput")
    w_sh = nc.dram_tensor("w_sh", [E, IN, CS], F32, kind="ExternalInput")
    b_sh = nc.dram_tensor("b_sh", [1, E * CS], F32, kind="ExternalInput")
    wg_s = nc.dram_tensor("wg_s", [OUT, E], F32, kind="ExternalInput")  # Wg/T
    bg_r = nc.dram_tensor("bg_r", [1, E], F32, kind="ExternalInput")
    ident = nc.dram_tensor("ident", [128, 128], F32, kind="ExternalInput")
    identr = nc.dram_tensor("identr", [128, 128], F32R, kind="ExternalInput")
    lsl = nc.dram_tensor("lsl", [128, 128], F32, kind="ExternalInput")
    bcast16 = nc.dram_tensor("bcast16", [NT, NT * 128], F32, kind="ExternalInput")
    iota_e = nc.dram_tensor("iota_e", [128, E], F32, kind="ExternalInput")
    tokid16 = nc.dram_tensor("tokid16", [128, NT * 16], U32, kind="ExternalInput")

    out_slots = nc.dram_tensor("out_slots", [SLOTS, CS], BF16, kind="ExternalOutput")
    top1_out = nc.dram_tensor("top1_out", [B, 1], U32, kind="ExternalOutput")

    with tile.TileContext(nc) as tc:
        with (
            tc.tile_pool(name="consts", bufs=1) as cpool,
            tc.tile_pool(name="dram", bufs=1, space="DRAM") as dpool,
            tc.tile_pool(name="wp", bufs=4) as wpool,
            tc.tile_pool(name="gat", bufs=4) as gpool,
            tc.tile_pool(name="gat1", bufs=1) as g1pool,
            # PSUM budget (8 banks): tp x2 + tpg x2 + gps x2 + po x2
            tc.tile_pool(name="gps", bufs=2, space="PSUM") as gpsum,
            tc.tile_pool(name="gpsg", bufs=2, space="PSUM") as gpsumg,
            tc.tile_pool(name="gps1", bufs=2, space="PSUM") as gpsum1,
            tc.tile_pool(name="rout", bufs=1) as r1pool,
            tc.tile_pool(name="scr", bufs=4) as spool,
            tc.tile_pool(name="xp", bufs=3) as xpool,
            tc.tile_pool(name="psl", bufs=1) as ppool,
            tc.tile_pool(name="op", bufs=3) as opool,
            tc.tile_pool(name="ops", bufs=2, space="PSUM") as opsum,
        ):
            # ---- W-slot dummies: hold the 4 W stream slots until late
            # gating chunks release them (keeps phase-1 HBM for t) ---------
            wdum = []
            for i in range(4):
                dm = wpool.tile([1, 1], F32, tag="wt", name=f"wdum{i}")
                nc.vector.memset(dm[:], 0.0)
                wdum.append(dm)

            # ---- constants resident in SBUF for the whole kernel ---------
            ident_sb = cpool.tile([128, 128], F32)
            nc.scalar.dma_start(ident_sb[:], ident[:, :])
            identr_sb = cpool.tile([128, 128], F32R)
            nc.scalar.dma_start(identr_sb[:], identr[:, :])
            lsl_sb = cpool.tile([128, 128], F32)
            nc.scalar.dma_start(lsl_sb[:], lsl[:, :])
            bcast16_sb = cpool.tile([NT, NT * 128], F32)
            nc.scalar.dma_start(bcast16_sb[:], bcast16[:, :])
            iota_e_sb = cpool.tile([128, E], F32)
            nc.scalar.dma_start(iota_e_sb[:], iota_e[:, :])
            tokid16_sb = cpool.tile([128, NT * 16], U32)
            nc.scalar.dma_start(tokid16_sb[:], tokid16[:, :])
            ones_sb = cpool.tile([128, 256], F32)
            nc.vector.memset(ones_sb[:], 1.0)
            # Wg/T laid out [128, 32*E]: wg_sb[p, kt*E+e] = Wg[kt*128+p, e]
            wg_sb = cpool.tile([128, (OUT // 128) * E], F32)
            nc.scalar.dma_start(
                wg_sb[:].rearrange("p (k e) -> p k e", e=E),
                wg_s[:, :].rearrange("(k p) e -> p k e", p=128),
            )
            bg_sb = cpool.tile([1, E], F32)
            nc.scalar.dma_start(bg_sb[:], bg_r[:, :])
            b_sb = cpool.tile([1, E * CS], F32)
            nc.scalar.dma_start(b_sb[:], b_sh[:, :])
            sent_sb = cpool.tile([128, SLOTS // 128 * 16], U32)
            nc.vector.memset(sent_sb[:], SENTINEL)

            # DRAM scratch
            top1_loc = dpool.tile([BS, 1], U32, name="t1loc")
            all_top1 = dpool.tile([B, 1], U32, name="allt1")
            perm = dpool.tile([SLOTS, 16], U32)

            # sentinel-init perm so padded slots drop their gather traffic
            nc.scalar.dma_start(
                perm[:, :].rearrange("(a p) n -> p a n", p=128),
                sent_sb[:].rearrange("p (a n) -> p a n", n=16),
            )

            # ================= phase 1: gating ============================
            qeng = [nc.scalar, nc.sync]
            ci = 0
            for tt in range(NTT):
                gps = gpsum1.tile([E, 128], F32, tag="gps")
                for dc in range(NDC):
                    chunk = gpool.tile([128, T, DC], F32, tag="tchunk")
                    qeng[ci % 2].dma_start(
                        chunk[:],
                        t_sh[tt * 128 : (tt + 1) * 128, :, dc * DC : (dc + 1) * DC],
                    )
                    # tree-reduce over T=8 into chunk[:, 0, :] -- exact f32;
                    # nc.any lets the scheduler spread adds over DVE+ACT
                    cf = chunk[:].rearrange("p t d -> p (t d)")
                    nc.any.tensor_add(
                        cf[:, 0 : 4 * DC], cf[:, 0 : 4 * DC], cf[:, 4 * DC : 8 * DC]
                    )
                    nc.any.tensor_add(
                        cf[:, 0 : 2 * DC], cf[:, 0 : 2 * DC], cf[:, 2 * DC : 4 * DC]
                    )
                    nc.any.tensor_add(
                        cf[:, 0:DC], cf[:, 0:DC], cf[:, DC : 2 * DC]
                    )
                    for k in range(DC // 128):
                        kt = dc * (DC // 128) + k
                        ptr = gpsum.tile([128, 128], F32, tag="tp")
                        nc.tensor.transpose(
                            ptr[:],
                            chunk[:, 0, k * 128 : (k + 1) * 128],
                            ident_sb[:, :],
                        )
                        tst = gpool.tile([128, 128], F32, tag="tsT", bufs=6)
                        nc.any.tensor_copy(tst[:], ptr[:])
                        nc.tensor.matmul(
                            gps[:],
                            lhsT=wg_sb[:, kt * E : (kt + 1) * E],
                            rhs=tst[:],
                            start=(kt == 0),
                            stop=False,
                        )
                    # release one W stream slot per late chunk (last 4):
                    # one op reading BOTH the dummy and this chunk ties the
                    # dummy's release (and thus the W load) to t progress
                    if ci >= NCH - 4:
                        di = ci - (NCH - 4)
                        scr = spool.tile([1, 1], F32, tag="scr")
                        nc.vector.tensor_add(
                            scr[:], wdum[di][:], chunk[0:1, 0, 0:1]
                        )
                    ci += 1
                nc.tensor.matmul(
                    gps[:],
                    lhsT=bg_sb[0:1, :],
                    rhs=ones_sb[0:1, 0:128],
                    start=False,
                    stop=True,
                )
                gT_sb = gpool.tile([E, 128], F32, tag="gT")
                nc.vector.tensor_copy(gT_sb[:], gps[:])
                gp = gpsumg.tile([128, E], F32, tag="tpg")
                nc.tensor.transpose(gp[:], gT_sb[:], ident_sb[0:E, 0:E])
                gate_sb = gpool.tile([128, E], F32, tag="gate")
                nc.vector.tensor_copy(gate_sb[:], gp[:])
                mxv = gpool.tile([128, 8], F32, tag="mxv")
                mxi = gpool.tile([128, 8], U32, tag="mxi")
                nc.vector.max_with_indices(mxv[:], mxi[:], gate_sb[:])
                nc.sync.dma_start(
                    top1_loc[tt * 128 : (tt + 1) * 128, :], mxi[:, 0:1]
                )

            # ================= phase 2: one AllGather =====================
            nc.gpsimd.collective_compute(
                "AllGather",
                mybir.AluOpType.bypass,
                replica_groups=[list(range(NCORES))],
                ins=[top1_loc[:].opt()],
                outs=[all_top1[:].opt()],
            )
            nc.sync.dma_start(top1_out[:, :], all_top1[:, :])

            # ================= phase 3: slot assignment (batched) =========
            tb_all = r1pool.tile([128, NT], U32)
            nc.scalar.dma_start(
                tb_all[:],
                all_top1[:, :].rearrange("(j p) one -> p (j one)", p=128),
            )
            t1f_all = r1pool.tile([128, NT], F32)
            nc.vector.tensor_copy(t1f_all[:], tb_all[:])
            oh_all = r1pool.tile([128, NT * E], F32)
            for i in range(NT):
                nc.vector.tensor_tensor(
                    out=oh_all[:, i * E : (i + 1) * E],
                    in0=t1f_all[:, i : i + 1].to_broadcast([128, E]),
                    in1=iota_e_sb[:],
                    op=mybir.AluOpType.is_equal,
                )
            # per-tile expert counts -> one psum row [1, NT*E]
            pcnt = gpsumg.tile([1, NT * E], F32, tag="tpg")
            nc.tensor.matmul(
                pcnt[:], lhsT=ones_sb[0:128, 0:1], rhs=oh_all[:],
                start=True, stop=True,
            )
            cnt_sb = r1pool.tile([1, NT * E], F32)
            nc.vector.tensor_copy(cnt_sb[:], pcnt[:])
            # counts2d [NT, E] via E strided mini-transposes
            pc2 = gpsumg.tile([NT, E], F32, tag="tpg")
            for e in range(E):
                nc.tensor.transpose(
                    pc2[:, e : e + 1],
                    cnt_sb[0:1, :].rearrange("one (i e) -> one i e", e=E)[:, :, e],
                    ident_sb[0:1, 0:1],
                )
            c2_sb = r1pool.tile([NT, E], F32)
            nc.vector.tensor_copy(c2_sb[:], pc2[:])
            # exclusive tile-base prefix: base2[i,e] = sum_{j<i} c2[j,e]
            pb2 = gpsumg.tile([NT, E], F32, tag="tpg")
            nc.tensor.matmul(
                pb2[:], lhsT=lsl_sb[0:NT, 0:NT], rhs=c2_sb[:],
                start=True, stop=True,
            )
            b2_sb = r1pool.tile([NT, E], F32)
            nc.vector.tensor_copy(b2_sb[:], pb2[:])

            # rank = within-tile exclusive prefix (one lsl matmul) + tile
            # base (per-tile partition-broadcast matmuls), separate psums
            pr1 = gpsumg.tile([128, NT * E], F32, tag="tpg")
            nc.tensor.matmul(
                pr1[:], lhsT=lsl_sb[:], rhs=oh_all[:],
                start=True, stop=True,
            )
            pr2 = gpsumg.tile([128, NT * E], F32, tag="tpg")
            for i in range(NT):
                nc.tensor.matmul(
                    pr2[:, i * E : (i + 1) * E],
                    lhsT=bcast16_sb[:, i * 128 : (i + 1) * 128],
                    rhs=b2_sb[:],
                    start=True,
                    stop=True,
                )
            sel = r1pool.tile([128, NT * E], F32)
            nc.vector.tensor_copy(sel[:], pr1[:])
            nc.vector.tensor_add(sel[:], sel[:], pr2[:])
            nc.vector.tensor_mul(sel[:], sel[:], oh_all[:])
            rank_all = r1pool.tile([128, NT], F32)
            for i in range(NT):
                nc.vector.reduce_sum(
                    rank_all[:, i : i + 1],
                    sel[:, i * E : (i + 1) * E],
                    axis=mybir.AxisListType.X,
                )
            posf = r1pool.tile([128, NT], F32)
            nc.vector.tensor_scalar(
                posf[:], t1f_all[:], float(CAP), scalar2=None,
                op0=mybir.AluOpType.mult,
            )
            nc.vector.tensor_add(posf[:], posf[:], rank_all[:])
            posu = r1pool.tile([128, NT], U32)
            nc.vector.tensor_copy(posu[:], posf[:])
            # 16 back-to-back scatters: slot table perm[slot] = token id
            for i in range(NT):
                nc.gpsimd.indirect_dma_start(
                    out=perm[:, :],
                    out_offset=IndirectOffsetOnAxis(ap=posu[:, i : i + 1], axis=0),
                    in_=tokid16_sb[:, i * 16 : (i + 1) * 16],
                    in_offset=None,
                    bounds_check=SLOTS - 1,
                    oob_is_err=False,
                )

            # ================= phase 4: expert matmul =====================
            # one combined pslice load: [p, a, n] with slot = a*128+p
            pslice = ppool.tile([128, (SLOTS // 128) * 16], U32)
            nc.scalar.dma_start(
                pslice[:].rearrange("p (a n) -> p a n", n=16),
                perm[:, :].rearrange("(a p) n -> p a n", p=128),
            )

            wts = []

            def load_w(e):
                wt = wpool.tile([128, NKX * CS], BF16, tag="wt", name=f"wt{e}")
                nc.gpsimd.dma_start(
                    wt[:].rearrange("p (k n) -> p k n", k=NKX),
                    w_sh[e].rearrange("(k p) n -> p k n", p=128),
                )
                wts.append(wt)

            def gather_x(e):
                xg = xpool.tile([128, (CAP // 128) * IN], F32R, tag="xg")
                for mt in range(CAP // 128):
                    a = e * (CAP // 128) + mt
                    nc.gpsimd.indirect_dma_start(
                        out=xg[:, mt * IN : (mt + 1) * IN],
                        out_offset=None,
                        in_=x_full[:, :],
                        in_offset=IndirectOffsetOnAxis(
                            ap=pslice[:, a * 16 : a * 16 + 1], axis=0
                        ),
                        bounds_check=B - 1,
                        oob_is_err=False,
                    )
                return xg

            # prologue: first 4 W loads (gated by the dummy slots) + 2 gathers
            for e in range(4):
                load_w(e)
            xgs = {0: gather_x(0), 1: gather_x(1)}

            for e in range(E):
                wt = wts[e]
                xg = xgs.pop(e)
                xgT = xpool.tile([128, (CAP // 128) * IN], BF16, tag="xgT")
                for mt in range(CAP // 128):
                    for k in range(NKX):
                        ptx = gpsum.tile([128, 128], F32R, tag="tp")
                        nc.tensor.transpose(
                            ptx[:],
                            xg[:, (mt * NKX + k) * 128 : (mt * NKX + k + 1) * 128],
                            identr_sb[:, :],
                        )
                        nc.any.tensor_copy(
                            xgT[:, (mt * NKX + k) * 128 : (mt * NKX + k + 1) * 128],
                            ptx[:],
                        )
                ot = opool.tile([128, (CAP // 128) * CS], BF16, tag="ot")
                for mt in range(CAP // 128):
                    po = opsum.tile([128, CS], F32, tag="po")
                    for k in range(NKX):
                        nc.tensor.matmul(
                            po[:],
                            lhsT=xgT[:, (mt * NKX + k) * 128 : (mt * NKX + k + 1) * 128],
                            rhs=wt[:, k * CS : (k + 1) * CS],
                            start=(k == 0),
                            stop=False,
                        )
                    nc.tensor.matmul(
                        po[:],
                        lhsT=ones_sb[0:1, 0:128],
                        rhs=b_sb[0:1, e * CS : (e + 1) * CS],
                        start=False,
                        stop=True,
                    )
                    nc.any.tensor_copy(ot[:, mt * CS : (mt + 1) * CS], po[:])
                # issue next gather / W load ahead of this expert's writeback
                if e + 2 < E:
                    xgs[e + 2] = gather_x(e + 2)
                if e + 4 < E:
                    load_w(e + 4)
                # contiguous slot-order writeback on the idle HWDGE queue
                nc.sync.dma_start(
                    out_slots[e * CAP : (e + 1) * CAP, :].rearrange(
                        "(m p) n -> p m n", p=128
                    ),
                    ot[:].rearrange("p (m n) -> p m n", n=CS),
                )

    nc.compile()
    return nc


def make_in_maps(inputs: dict) -> list[dict]:
    x = np.ascontiguousarray(np.asarray(inputs["x"], dtype=np.float32))
    t = np.ascontiguousarray(np.asarray(inputs["t"], dtype=np.float32))
    W = np.ascontiguousarray(np.asarray(inputs["W"], dtype=np.float32))
    b = np.ascontiguousarray(np.asarray(inputs["b"], dtype=np.float32))
    Wg = np.ascontiguousarray(np.asarray(inputs["Wg"], dtype=np.float32))
    bg = np.ascontiguousarray(np.asarray(inputs["bg"], dtype=np.float32))

    x2 = np.ascontiguousarray(x[:, 0, :])                       # [B, IN]
    ident = np.eye(128, dtype=np.float32)
    lsl = np.triu(np.ones((128, 128), np.float32), k=1)          # lsl[r,c]=1 iff r<c
    # bcast16[j, i*128+p] = 1 iff i == j  (base-row broadcast selector)
    bcast16 = np.zeros((NT, NT * 128), np.float32)
    for i in range(NT):
        bcast16[i, i * 128 : (i + 1) * 128] = 1.0
    iota_e = np.tile(np.arange(E, dtype=np.float32)[None, :], (128, 1))
    # tokid16[p, j*16+r] = global token id j*128+p (replicated 16x per row)
    tokid16 = np.zeros((128, NT * 16), np.uint32)
    for j in range(NT):
        tokid16[:, j * 16 : (j + 1) * 16] = (
            j * 128 + np.arange(128, dtype=np.uint32)[:, None]
        )

    in_maps = []
    for c in range(NCORES):
        cs = slice(c * CS, (c + 1) * CS)
        in_maps.append({
            "t_sh": np.ascontiguousarray(t[c * BS : (c + 1) * BS]),
            "x_full": x2,
            "w_sh": np.ascontiguousarray(W[:, :, cs]),
            "b_sh": np.ascontiguousarray(b[:, cs]).reshape(1, E * CS),
            "wg_s": np.ascontiguousarray(Wg / float(T)),
            "bg_r": bg.reshape(1, E),
            "ident": ident,
            "identr": ident,
            "lsl": lsl,
            "bcast16": bcast16,
            "iota_e": iota_e,
            "tokid16": tokid16,
        })
    return in_maps


def assemble_output(per_core_results: list[dict]) -> np.ndarray:
    top1 = np.asarray(per_core_results[0]["top1_out"]).reshape(B).astype(np.int64)
    # recompute slot(token) exactly as the device did (stable within-expert rank)
    rank = np.zeros(B, dtype=np.int64)
    counts = np.zeros(E, dtype=np.int64)
    for i in range(B):
        e = top1[i]
        rank[i] = counts[e]
        counts[e] += 1
    assert counts.max() <= CAP, f"expert overflow: {counts}"
    slot = top1 * CAP + rank
    out = np.empty((B, 1, OUT), dtype=np.float32)
    for c in range(NCORES):
        osl = np.asarray(per_core_results[c]["out_slots"]).astype(np.float32)
        out[:, 0, c * CS : (c + 1) * CS] = osl[slot]
    return out


_NC_CACHE = {}


def kernel(**inputs) -> np.ndarray:
    if "nc" not in _NC_CACHE:
        _NC_CACHE["nc"] = build_kernel()
    nc = _NC_CACHE["nc"]
    in_maps = make_in_maps(inputs)
    res = run_bass_kernel_spmd(nc, in_maps, core_ids=list(range(NCORES)))
    return assemble_output(res.results)
